# revision 1
# baseline (speedup 1.0000x reference)
"""DGCNN (gnn_message_passing) Trainium2 Bass kernel.

Strategy (data-parallel over graphs, 8 graphs per NeuronCore):
  - Host builds, per graph, the dense normalized propagation operator
    S[d, s] = (mult(s->d) + I) / deg_out[d]  (512x512 f32), shipped
    transposed as 4 chunks of [128, 512].
  - On device, each GCN layer is two matmul stages per graph:
      lin  = h @ W          (node-major, PE, 4 chunks)
      msgT = lin^T-chunks stationary x S^T chunks -> [128f, 512d] PSUM
      h'   = tanh(msgT)     (ACT, feature-major back to SBUF)
  - Sort-pooling (top-64 by last channel, stable ties) is computed exactly
    with comparison matrices on DVE:
      rank[i] = #{j: v[j] > v[i]} + #{j < i: v[j] == v[i]}
    then a 0/1 selection matrix PT[i,k] = (rank[i] == k) applied via PE
    matmuls (with PE-transposed node-major h chunks).
  - Conv1d/maxpool/conv2/dense head all on device as small matmuls; final
    2-class softmax via sigmoid of logit differences.

Self-contained: hardcodes all shapes; no reads of /root/problem files.
"""

import sys

if "/opt/trn_rl_repo" not in sys.path:
    sys.path.insert(0, "/opt/trn_rl_repo")

import numpy as np

import concourse.bacc as bacc
import concourse.mybir as mybir
import concourse.tile as tile
from concourse.bass_utils import run_bass_kernel_spmd

F32 = mybir.dt.float32
F32R = mybir.dt.float32r

NUM_GRAPHS = 64
NPG = 512  # nodes per graph
N_TOTAL = NUM_GRAPHS * NPG
EMB = 128
DIMF = 128
NLAYERS = 4
K = 64
NCORES = 8
GPC = NUM_GRAPHS // NCORES  # graphs per core = 8
NLOC = GPC * NPG  # local nodes = 4096
LATENT = NLAYERS * DIMF + 1  # 513
DD = (K - 2) // 2 + 1  # 32
CONV2_LEN = DD - 5 + 1  # 28

# Matmul dtype for the GCN stages: F32 (safe) or F32R (fast, 11-bit operand
# mantissa). Switched by _build(); default chosen in kernel().
_NC_CACHE = {}


def _round_fp32r(x):
    """Round fp32 array to the fp32r grid (RNE to 11 mantissa bits)."""
    u = np.ascontiguousarray(x, dtype=np.float32).view(np.uint32)
    bias = np.uint32(0x7FF) + ((u >> np.uint32(12)) & np.uint32(1))
    r = ((u + bias) >> np.uint32(12)) << np.uint32(12)
    return r.view(np.float32)


def _build(dt_mm, with_bias, debug):
    """Trace + compile the per-core Bass program (same on all 8 cores)."""
    nc = bacc.Bacc("TRN2", target_bir_lowering=False, debug=False,
                   num_devices=NCORES)
    DT = dt_mm

    # ---- per-core DRAM I/O ----
    H0T = nc.dram_tensor("H0T", [128, NLOC], DT, kind="ExternalInput")
    STD = nc.dram_tensor("STD", [GPC, 4, 128, NPG], DT, kind="ExternalInput")
    WC = nc.dram_tensor("WC", [NLAYERS, 128, 128], DT, kind="ExternalInput")
    W5 = nc.dram_tensor("W5", [128, 1], DT, kind="ExternalInput")
    IDN = nc.dram_tensor("IDN", [128, 128], DT, kind="ExternalInput")
    W1A = nc.dram_tensor("W1A", [4, 128, 16], F32, kind="ExternalInput")
    W1B = nc.dram_tensor("W1B", [1, 16], F32, kind="ExternalInput")
    W2T = nc.dram_tensor("W2T", [5, 16, 32], F32, kind="ExternalInput")
    D1R = nc.dram_tensor("D1R", [32, CONV2_LEN * 32], F32, kind="ExternalInput")
    D2 = nc.dram_tensor("D2", [32, 2], F32, kind="ExternalInput")
    DIFF = nc.dram_tensor("DIFF", [2, 2], F32, kind="ExternalInput")
    B1 = nc.dram_tensor("B1", [16, 1], F32, kind="ExternalInput")
    B2 = nc.dram_tensor("B2", [32, 1], F32, kind="ExternalInput")
    BD1 = nc.dram_tensor("BD1", [32, 1], F32, kind="ExternalInput")
    BD2 = nc.dram_tensor("BD2", [2, 1], F32, kind="ExternalInput")
    KI = nc.dram_tensor("KI", [128, K], F32, kind="ExternalInput")
    MJ = nc.dram_tensor("MJ", [4, 128, NPG], F32, kind="ExternalInput")
    if with_bias:
        ONE = nc.dram_tensor("ONE", [1, 128], DT, kind="ExternalInput")
        BCV = nc.dram_tensor("BCV", [NLAYERS, 1, 128], DT, kind="ExternalInput")
        B5V = nc.dram_tensor("B5V", [1, 1], DT, kind="ExternalInput")
    OUT = nc.dram_tensor("OUT", [2, GPC], F32, kind="ExternalOutput")
    if debug:
        DBG_H = nc.dram_tensor("DBG_H", [NLAYERS, 128, NLOC], F32,
                               kind="ExternalOutput")
        DBG_H5 = nc.dram_tensor("DBG_H5", [GPC, 1, NPG], F32,
                                kind="ExternalOutput")
        DBG_RANK = nc.dram_tensor("DBG_RANK", [GPC, 128, 4], F32,
                                  kind="ExternalOutput")
        DBG_POOL = nc.dram_tensor("DBG_POOL", [GPC, 128, 4 * K], F32,
                                  kind="ExternalOutput")
        DBG_POOL5 = nc.dram_tensor("DBG_POOL5", [GPC, 1, K], F32,
                                   kind="ExternalOutput")
        DBG_Y2 = nc.dram_tensor("DBG_Y2", [32, GPC * CONV2_LEN], F32,
                                kind="ExternalOutput")

    TANH = mybir.ActivationFunctionType.Tanh
    RELU = mybir.ActivationFunctionType.Relu
    SIGM = mybir.ActivationFunctionType.Sigmoid
    ADD = mybir.AluOpType.add
    MULT = mybir.AluOpType.mult
    MAX = mybir.AluOpType.max
    IS_GT = mybir.AluOpType.is_gt
    IS_EQ = mybir.AluOpType.is_equal

    with tile.TileContext(nc) as tc:
        with (
            tc.tile_pool(name="const", bufs=1) as cp,
            tc.tile_pool(name="hs", bufs=1) as hp,
            tc.tile_pool(name="st", bufs=1) as stp,
            tc.tile_pool(name="lin", bufs=8) as linp,
            tc.tile_pool(name="sc", bufs=2) as scp,
            tc.tile_pool(name="sm", bufs=2) as smp,
            tc.tile_pool(name="xs", bufs=4) as xsp,
            tc.tile_pool(name="ps512", bufs=3, space="PSUM") as ps5,
            tc.tile_pool(name="ps128", bufs=5, space="PSUM") as ps1,
        ):
            # ---- constant / weight loads ----
            st_sb = []
            for g in range(GPC):
                t = stp.tile([128, 4 * NPG], DT, tag=f"st{g}")
                for c in range(4):
                    nc.sync.dma_start(t[:, c * NPG:(c + 1) * NPG],
                                      STD[g, c, :, :])
                st_sb.append(t)
            h0 = hp.tile([128, NLOC], DT, tag="h0")
            for c in range(8):
                nc.sync.dma_start(h0[:, c * 512:(c + 1) * 512],
                                  H0T[:, c * 512:(c + 1) * 512])
            wc_sb = cp.tile([128, NLAYERS * 128], DT, tag="wc")
            for l in range(NLAYERS):
                nc.sync.dma_start(wc_sb[:, l * 128:(l + 1) * 128], WC[l, :, :])
            w5_sb = cp.tile([128, 1], DT, tag="w5")
            nc.sync.dma_start(w5_sb[:], W5[:])
            id_sb = cp.tile([128, 128], DT, tag="idn")
            nc.sync.dma_start(id_sb[:], IDN[:])
            w1a_sb = cp.tile([128, 64], F32, tag="w1a")
            for c in range(4):
                nc.sync.dma_start(w1a_sb[:, c * 16:(c + 1) * 16], W1A[c, :, :])
            w1b_sb = cp.tile([1, 16], F32, tag="w1b")
            nc.sync.dma_start(w1b_sb[:], W1B[:])
            w2_sb = cp.tile([16, 160], F32, tag="w2t")
            for t5 in range(5):
                nc.sync.dma_start(w2_sb[:, t5 * 32:(t5 + 1) * 32],
                                  W2T[t5, :, :])
            d1_sb = cp.tile([32, CONV2_LEN * 32], F32, tag="d1r")
            nc.sync.dma_start(d1_sb[:], D1R[:])
            d2_sb = cp.tile([32, 2], F32, tag="d2")
            nc.sync.dma_start(d2_sb[:], D2[:])
            diff_sb = cp.tile([2, 2], F32, tag="diff")
            nc.sync.dma_start(diff_sb[:], DIFF[:])
            b1_sb = cp.tile([16, 1], F32, tag="b1")
            nc.sync.dma_start(b1_sb[:], B1[:])
            b2_sb = cp.tile([32, 1], F32, tag="b2")
            nc.sync.dma_start(b2_sb[:], B2[:])
            bd1_sb = cp.tile([32, 1], F32, tag="bd1")
            nc.sync.dma_start(bd1_sb[:], BD1[:])
            bd2_sb = cp.tile([2, 1], F32, tag="bd2")
            nc.sync.dma_start(bd2_sb[:], BD2[:])
            ki_sb = cp.tile([128, K], F32, tag="ki")
            nc.sync.dma_start(ki_sb[:], KI[:])
            mj_sb = cp.tile([128, 4 * NPG], F32, tag="mj")
            for c in range(4):
                nc.sync.dma_start(mj_sb[:, c * NPG:(c + 1) * NPG], MJ[c, :, :])
            if with_bias:
                one_sb = cp.tile([1, 128], DT, tag="one")
                nc.sync.dma_start(one_sb[:], ONE[:])
                bcv_sb = []
                for l in range(NLAYERS):
                    t = cp.tile([1, 128], DT, tag=f"bcv{l}")
                    nc.sync.dma_start(t[:], BCV[l, :, :])
                    bcv_sb.append(t)
                b5v_sb = cp.tile([1, 1], DT, tag="b5v")
                nc.sync.dma_start(b5v_sb[:], B5V[:])
            y2all = cp.tile([32, GPC * CONV2_LEN], F32, tag="y2all")

            # ---- GCN layers 1..4 ----
            h_prev = h0
            h_layers = []
            for l in range(NLAYERS):
                h_next = hp.tile([128, NLOC], DT, tag=f"h{l + 1}")
                for g in range(GPC):
                    lins = []
                    for cc in range(4):
                        ch = 4 * g + cc
                        lp = ps1.tile([128, 128], F32, tag="ps128")
                        nc.tensor.matmul(
                            lp[:], h_prev[:, ch * 128:(ch + 1) * 128],
                            wc_sb[:, l * 128:(l + 1) * 128],
                            start=True, stop=not with_bias)
                        if with_bias:
                            nc.tensor.matmul(lp[:], one_sb[:], bcv_sb[l][:],
                                             start=False, stop=True)
                        ln = linp.tile([128, 128], DT, tag="lin")
                        nc.vector.tensor_copy(ln[:], lp[:])
                        lins.append(ln)
                    sp = ps5.tile([128, NPG], F32, tag="ps512")
                    for cc in range(4):
                        nc.tensor.matmul(
                            sp[:], lins[cc][:],
                            st_sb[g][:, cc * NPG:(cc + 1) * NPG],
                            start=(cc == 0), stop=(cc == 3))
                    nc.scalar.activation(
                        h_next[:, g * NPG:(g + 1) * NPG], sp[:], TANH)
                h_layers.append(h_next)
                h_prev = h_next
            if debug:
                for l in range(NLAYERS):
                    nc.sync.dma_start(DBG_H[l, :, :], h_layers[l][:])

            # ---- per-graph: last layer, rank, pooling, head ----
            for g in range(GPC):
                # layer 5: lin5 = h4 @ W_last (node-major columns)
                l5p = ps1.tile([128, 4], F32, tag="ps128")
                for cc in range(4):
                    ch = 4 * g + cc
                    nc.tensor.matmul(
                        l5p[:, cc:cc + 1],
                        h_prev[:, ch * 128:(ch + 1) * 128], w5_sb[:],
                        start=True, stop=not with_bias)
                    if with_bias:
                        nc.tensor.matmul(l5p[:, cc:cc + 1], one_sb[:],
                                         b5v_sb[:], start=False, stop=True)
                lin5 = smp.tile([128, 4], DT, tag="lin5")
                nc.vector.tensor_copy(lin5[:], l5p[:])
                m5p = ps5.tile([1, NPG], F32, tag="ps512")
                for cc in range(4):
                    nc.tensor.matmul(m5p[:], lin5[:, cc:cc + 1],
                                     st_sb[g][:, cc * NPG:(cc + 1) * NPG],
                                     start=(cc == 0), stop=(cc == 3))
                h5r = smp.tile([1, NPG], F32, tag="h5r")
                nc.scalar.activation(h5r[:], m5p[:], TANH)
                if debug:
                    nc.sync.dma_start(DBG_H5[g, :, :], h5r[:])

                # v broadcast (exact copies)
                vb = scp.tile([128, NPG], F32, tag="vb")
                nc.gpsimd.partition_broadcast(vb[:], h5r[0:1, :])
                vcol = smp.tile([128, 4], F32, tag="vcol")
                for cc in range(4):
                    nc.sync.dma_start(vcol[:, cc:cc + 1],
                                      h5r[0:1, cc * 128:(cc + 1) * 128])

                # rank[i] = #{v[j] > v[i]} + #{j<i: v[j] == v[i]}
                rank = smp.tile([128, 4], F32, tag="rank")
                for cc in range(4):
                    t1 = scp.tile([128, NPG], F32, tag="t1")
                    ra = smp.tile([128, 2], F32, tag="ra")
                    nc.vector.tensor_scalar(
                        out=t1[:], in0=vb[:], scalar1=vcol[:, cc:cc + 1],
                        scalar2=None, op0=IS_GT, op1=ADD,
                        accum_out=ra[:, 0:1])
                    t2 = scp.tile([128, NPG], F32, tag="t2")
                    nc.vector.scalar_tensor_tensor(
                        out=t2[:], in0=vb[:], scalar=vcol[:, cc:cc + 1],
                        in1=mj_sb[:, cc * NPG:(cc + 1) * NPG],
                        op0=IS_EQ, op1=MULT, accum_out=ra[:, 1:2])
                    nc.vector.tensor_tensor(
                        out=rank[:, cc:cc + 1], in0=ra[:, 0:1],
                        in1=ra[:, 1:2], op=ADD)
                if debug:
                    nc.sync.dma_start(DBG_RANK[g, :, :], rank[:])

                # selection matrix PT[i, k] = (rank[i] == k)
                ptt = scp.tile([128, 4 * K], F32, tag="pt")
                for cc in range(4):
                    nc.vector.tensor_scalar(
                        out=ptt[:, cc * K:(cc + 1) * K], in0=ki_sb[:],
                        scalar1=rank[:, cc:cc + 1], scalar2=None, op0=IS_EQ)

                # pooled^T[f, k] for layers 1..4 via PE transpose + matmul
                pooledT = scp.tile([128, 4 * K], F32, tag="pooled")
                for l in range(NLAYERS):
                    pp = ps1.tile([128, K], F32, tag="ps128")
                    for cc in range(4):
                        ch = 4 * g + cc
                        xp = ps1.tile([128, 128], F32, tag="ps128")
                        nc.tensor.transpose(
                            xp[:], h_layers[l][:, ch * 128:(ch + 1) * 128],
                            id_sb[:])
                        xt = xsp.tile([128, 128], F32, tag="x")
                        nc.vector.tensor_copy(xt[:], xp[:])
                        nc.tensor.matmul(pp[:], xt[:],
                                         ptt[:, cc * K:(cc + 1) * K],
                                         start=(cc == 0), stop=(cc == 3))
                    nc.vector.tensor_copy(pooledT[:, l * K:(l + 1) * K], pp[:])
                p5p = ps1.tile([1, K], F32, tag="ps128")
                for cc in range(4):
                    nc.tensor.matmul(p5p[:], vcol[:, cc:cc + 1],
                                     ptt[:, cc * K:(cc + 1) * K],
                                     start=(cc == 0), stop=(cc == 3))
                pool5 = smp.tile([1, K], F32, tag="pool5")
                nc.vector.tensor_copy(pool5[:], p5p[:])
                if debug:
                    nc.sync.dma_start(DBG_POOL[g, :, :], pooledT[:])
                    nc.sync.dma_start(DBG_POOL5[g, :, :], pool5[:])

                # head: conv1 (1x513 stride-513) -> relu -> maxpool2
                y1p = ps1.tile([16, K], F32, tag="ps128")
                for cc in range(4):
                    nc.tensor.matmul(y1p[:], w1a_sb[:, cc * 16:(cc + 1) * 16],
                                     pooledT[:, cc * K:(cc + 1) * K],
                                     start=(cc == 0), stop=False)
                nc.tensor.matmul(y1p[:], w1b_sb[:], pool5[:],
                                 start=False, stop=True)
                y1 = smp.tile([16, K], F32, tag="y1")
                nc.scalar.activation(y1[:], y1p[:], RELU, bias=b1_sb[:, 0:1])
                mp = smp.tile([16, K // 2], F32, tag="mp")
                y1v = y1[:].rearrange("p (a b) -> p a b", b=2)
                nc.vector.tensor_tensor(out=mp[:], in0=y1v[:, :, 0:1],
                                        in1=y1v[:, :, 1:2], op=MAX)

                # conv2 (kernel 5) -> relu
                y2p = ps1.tile([32, CONV2_LEN], F32, tag="ps128")
                for t5 in range(5):
                    nc.tensor.matmul(y2p[:], w2_sb[:, t5 * 32:(t5 + 1) * 32],
                                     mp[:, t5:t5 + CONV2_LEN],
                                     start=(t5 == 0), stop=(t5 == 4))
                nc.scalar.activation(
                    y2all[:, g * CONV2_LEN:(g + 1) * CONV2_LEN], y2p[:],
                    RELU, bias=b2_sb[:, 0:1])

            if debug:
                nc.sync.dma_start(DBG_Y2[:], y2all[:])

            # ---- core-level dense tail (batched over the 8 graphs) ----
            h1p = ps1.tile([32, GPC], F32, tag="ps128")
            y2v = y2all[:].rearrange("p (g t) -> p g t", t=CONV2_LEN)
            for t5 in range(CONV2_LEN):
                nc.tensor.matmul(h1p[:], d1_sb[:, t5 * 32:(t5 + 1) * 32],
                                 y2v[:, :, t5:t5 + 1],
                                 start=(t5 == 0), stop=(t5 == CONV2_LEN - 1))
            h1s = smp.tile([32, GPC], F32, tag="h1s")
            nc.scalar.activation(h1s[:], h1p[:], RELU, bias=bd1_sb[:, 0:1])
            lgp = ps1.tile([2, GPC], F32, tag="ps128")
            nc.tensor.matmul(lgp[:], d2_sb[:], h1s[:], start=True, stop=True)
            lg = smp.tile([2, GPC], F32, tag="lg")
            nc.vector.tensor_scalar(out=lg[:], in0=lgp[:],
                                    scalar1=bd2_sb[:, 0:1], scalar2=None,
                                    op0=ADD)
            dfp = ps1.tile([2, GPC], F32, tag="ps128")
            nc.tensor.matmul(dfp[:], diff_sb[:], lg[:], start=True, stop=True)
            pr = smp.tile([2, GPC], F32, tag="pr")
            nc.scalar.activation(pr[:], dfp[:], SIGM)
            nc.sync.dma_start(OUT[:], pr[:])

    nc.compile()
    return nc


def _get_nc(dt_key, with_bias, debug):
    key = (dt_key, with_bias, debug)
    if key not in _NC_CACHE:
        dt_mm = F32R if dt_key == "f32r" else F32
        _NC_CACHE[key] = _build(dt_mm, with_bias, debug)
    return _NC_CACHE[key]


def prepare_host(inputs, dt_key):
    """All host-side index preprocessing + per-core input maps."""
    x = np.asarray(inputs["x"]).astype(np.int64)
    edge_index = np.asarray(inputs["edge_index"]).astype(np.int64)
    emb = np.ascontiguousarray(np.asarray(inputs["emb"], dtype=np.float32))
    W_convs = np.asarray(inputs["W_convs"], dtype=np.float32)
    b_convs = np.asarray(inputs["b_convs"], dtype=np.float32)
    W_last = np.asarray(inputs["W_last"], dtype=np.float32)
    b_last = np.asarray(inputs["b_last"], dtype=np.float32)
    conv1_w = np.asarray(inputs["conv1_w"], dtype=np.float32)
    conv1_b = np.asarray(inputs["conv1_b"], dtype=np.float32)
    conv2_w = np.asarray(inputs["conv2_w"], dtype=np.float32)
    conv2_b = np.asarray(inputs["conv2_b"], dtype=np.float32)
    d1_w = np.asarray(inputs["d1_w"], dtype=np.float32)
    d1_b = np.asarray(inputs["d1_b"], dtype=np.float32)
    d2_w = np.asarray(inputs["d2_w"], dtype=np.float32)
    d2_b = np.asarray(inputs["d2_b"], dtype=np.float32)

    src, dst = edge_index[0], edge_index[1]
    deg = (np.bincount(src, minlength=N_TOTAL) + 1).astype(np.float32)
    invdeg = (np.float32(1.0) / deg).astype(np.float32)
    gid = dst >> 9
    flat = (gid * NPG + (dst & 511)) * NPG + (src & 511)
    A = np.bincount(flat, minlength=NUM_GRAPHS * NPG * NPG)
    A = A.astype(np.float32).reshape(NUM_GRAPHS, NPG, NPG)
    idx = np.arange(NPG)
    A[:, idx, idx] += 1.0
    S = A * invdeg.reshape(NUM_GRAPHS, NPG, 1)
    ST = np.ascontiguousarray(S.transpose(0, 2, 1)).reshape(
        NUM_GRAPHS, 4, 128, NPG)

    h0 = emb[x]  # [N, 128]

    rnd = _round_fp32r if dt_key == "f32r" else (lambda a: a)
    with_bias = bool(np.any(b_convs) or np.any(b_last))

    w1 = np.ascontiguousarray(conv1_w[:, 0, :].T)  # [513, 16]
    shared = {
        "WC": rnd(np.ascontiguousarray(W_convs)),
        "W5": rnd(np.ascontiguousarray(W_last)),
        "IDN": rnd(np.eye(128, dtype=np.float32)),
        "W1A": np.ascontiguousarray(w1[:512].reshape(4, 128, 16)),
        "W1B": np.ascontiguousarray(w1[512:513]),
        "W2T": np.ascontiguousarray(conv2_w.transpose(2, 1, 0)),
        "D1R": np.ascontiguousarray(d1_w.reshape(DD, CONV2_LEN * 32)
                                    .astype(np.float32)),
        "D2": np.ascontiguousarray(d2_w),
        "DIFF": np.array([[1.0, -1.0], [-1.0, 1.0]], dtype=np.float32),
        "B1": np.ascontiguousarray(conv1_b.reshape(16, 1)),
        "B2": np.ascontiguousarray(conv2_b.reshape(32, 1)),
        "BD1": np.ascontiguousarray(d1_b.reshape(32, 1)),
        "BD2": np.ascontiguousarray(d2_b.reshape(2, 1)),
        "KI": np.ascontiguousarray(
            np.broadcast_to(np.arange(K, dtype=np.float32), (128, K))),
        "MJ": np.ascontiguousarray(
            (np.arange(NPG)[None, None, :]
             < (np.arange(4)[:, None, None] * 128
                + np.arange(128)[None, :, None])).astype(np.float32)),
    }
    if with_bias:
        shared["ONE"] = rnd(np.ones((1, 128), dtype=np.float32))
        shared["BCV"] = rnd(np.ascontiguousarray(
            b_convs.reshape(NLAYERS, 1, 128)))
        shared["B5V"] = rnd(np.ascontiguousarray(b_last.reshape(1, 1)))

    in_maps = []
    for c in range(NCORES):
        h0c = np.ascontiguousarray(h0[c * NLOC:(c + 1) * NLOC].T)
        m = dict(shared)
        m["H0T"] = rnd(h0c)
        m["STD"] = rnd(np.ascontiguousarray(ST[c * GPC:(c + 1) * GPC]))
        in_maps.append(m)
    return in_maps, with_bias


def run(inputs, dt_key="f32", debug=False, **spmd_kwargs):
    in_maps, with_bias = prepare_host(inputs, dt_key)
    nc = _get_nc(dt_key, with_bias, debug)
    res = run_bass_kernel_spmd(nc, in_maps, core_ids=list(range(NCORES)),
                               **spmd_kwargs)
    out = np.empty((NUM_GRAPHS, 2), dtype=np.float32)
    for c in range(NCORES):
        out[c * GPC:(c + 1) * GPC, :] = res.results[c]["OUT"].T
    return out, res


def kernel(**inputs):
    out, _ = run(inputs, dt_key="f32")
    return out
